# revision 4
# baseline (speedup 1.0000x reference)
"""Distributed causal multi-head attention for Trainium2 (8 NeuronCores).

Problem: B=2, S=2048, d_model=1024, 16 heads x 64 dims, causal softmax attention.

Strategy (tensor-parallel over heads + all-to-all for output projection):
  - Each core owns 2 heads (128 of the 1024 QKV features).
  - Host pre-transposes x -> X^T [1024, 4096] and casts inputs to bf16, so all
    on-chip matmuls consume feature-on-partition ("transposed") activations
    directly with no on-chip transposes of x.
  - Per core: Q^T/K^T/V^T = W^T-shard @ X^T (+bias), attention per (batch, head)
    in S^T layout ([k-partitions, q-free]) with exp (no max subtraction; scores
    are O(1) so fp32 exp is safe), causal masking via a single 128x128 upper-
    triangular mask on diagonal tiles, and denominators via an appended ones
    column on V (PE computes the partition-dim sums for free).
  - Normalization pre-collective (per-head denominators), then one AllToAll per
    batch redistributes O^T from head-sharded to row-sharded; each core then
    computes its 2x256 output rows with the full Wo.
  - Output f32; host reassembles the full [2, 2048, 1024].
"""
import os
import sys

sys.path.insert(0, "/opt/trn_rl_repo")

import numpy as np
import ml_dtypes

from concourse import bacc, mybir, tile
from concourse.bass_utils import run_bass_kernel_spmd

BF16 = mybir.dt.bfloat16
F32 = mybir.dt.float32

B, S, DM = 2, 2048, 1024
H, DK = 16, 64
N_CORES = 8
FPC = 128          # features per core = 2 heads x 64
RPC = S // N_CORES  # output rows per core per batch = 256
NKT = S // 128      # k-tiles per batch = 16
NQC = S // 512      # q-chunks per batch = 4
SCALE = 1.0 / 8.0   # 1/sqrt(64)

_cache = {}


def _build():
    nc = bacc.Bacc("TRN2", target_bir_lowering=False, debug=False, num_devices=N_CORES)

    xt = nc.dram_tensor("xt", [DM, B * S], BF16, kind="ExternalInput")
    wq = nc.dram_tensor("wq", [DM, FPC], BF16, kind="ExternalInput")
    wk = nc.dram_tensor("wk", [DM, FPC], BF16, kind="ExternalInput")
    wv = nc.dram_tensor("wv", [DM, FPC], BF16, kind="ExternalInput")
    wo = nc.dram_tensor("wo", [DM, DM], BF16, kind="ExternalInput")
    bq = nc.dram_tensor("bq", [FPC, 1], F32, kind="ExternalInput")
    bk = nc.dram_tensor("bk", [FPC, 1], F32, kind="ExternalInput")
    bv = nc.dram_tensor("bv", [FPC, 1], F32, kind="ExternalInput")
    trimask = nc.dram_tensor("trimask", [128, 128], BF16, kind="ExternalInput")
    ident = nc.dram_tensor("ident", [128, 128], BF16, kind="ExternalInput")
    out_ext = nc.dram_tensor("out", [B, RPC, DM], F32, kind="ExternalOutput")

    EXP = mybir.ActivationFunctionType.Exp
    IDENT = mybir.ActivationFunctionType.Identity
    rg = [list(range(N_CORES))]

    with tile.TileContext(nc) as tc:
        with (
            tc.tile_pool(name="xtp", bufs=1) as xtp,
            tc.tile_pool(name="wts", bufs=1) as wts,
            tc.tile_pool(name="qkv", bufs=1) as qkvp,
            tc.tile_pool(name="vnat", bufs=1) as vnatp,
            tc.tile_pool(name="work", bufs=3) as work,
            tc.tile_pool(name="stage", bufs=2) as stagep,
            tc.tile_pool(name="outp", bufs=2) as outp,
            tc.tile_pool(name="psmm", bufs=2, space="PSUM") as psmm,
            tc.tile_pool(name="psS", bufs=2, space="PSUM") as psS,
            tc.tile_pool(name="psO", bufs=1, space="PSUM") as psO,
            tc.tile_pool(name="dram", bufs=1, space="DRAM") as dram,
        ):
            # ---------- load everything ----------
            xt_sb = []
            for kc in range(8):
                t = xtp.tile([128, B * S], BF16, tag=f"xt{kc}")
                nc.sync.dma_start(t[:], xt[kc * 128:(kc + 1) * 128, :])
                xt_sb.append(t)

            def load_w(w, name):
                tiles = []
                for kc in range(8):
                    t = wts.tile([128, FPC], BF16, tag=f"{name}{kc}")
                    nc.sync.dma_start(t[:], w[kc * 128:(kc + 1) * 128, :])
                    tiles.append(t)
                return tiles

            wq_sb = load_w(wq, "wq")
            wk_sb = load_w(wk, "wk")
            wv_sb = load_w(wv, "wv")
            wo_sb = []
            for kc in range(8):
                t = wts.tile([128, DM], BF16, tag=f"wo{kc}")
                nc.sync.dma_start(t[:], wo[kc * 128:(kc + 1) * 128, :])
                wo_sb.append(t)
            b_sb = {}
            for name, b in (("q", bq), ("k", bk), ("v", bv)):
                t = wts.tile([FPC, 1], F32, tag=f"b{name}")
                nc.sync.dma_start(t[:], b[:])
                b_sb[name] = t
            mask_sb = wts.tile([128, 128], BF16, tag="mask")
            nc.sync.dma_start(mask_sb[:], trimask[:])
            ident_sb = wts.tile([128, 128], BF16, tag="ident")
            nc.sync.dma_start(ident_sb[:], ident[:])

            # ---------- phase 1: QKV projections (transposed layout) ----------
            # projT [128 feat, 4096 rows] bf16 = sum_kc w[kc].T @ xt[kc]
            proj_sb = {}
            for name, w_tiles in (("q", wq_sb), ("k", wk_sb), ("v", wv_sb)):
                pt = qkvp.tile([128, B * S], BF16, tag=f"{name}T")
                proj_sb[name] = pt
                for rc in range(8):
                    ps = psmm.tile([128, 512], F32, tag="mm")
                    for kc in range(8):
                        nc.tensor.matmul(
                            ps[:], w_tiles[kc][:], xt_sb[kc][:, rc * 512:(rc + 1) * 512],
                            start=(kc == 0), stop=(kc == 7),
                        )
                    # bias-add + cast to bf16 on ScalarE
                    nc.scalar.activation(
                        pt[:, rc * 512:(rc + 1) * 512], ps[:], IDENT, bias=b_sb[name][:],
                    )
            qT, kT, vT = proj_sb["q"], proj_sb["k"], proj_sb["v"]

            # ---------- phase 2: V natural (+ones cols) via PE transpose ----------
            # v_nat[b][kt] [128 k, 130]: cols 0:64 head0 V, col 64 ones,
            #                            cols 65:129 head1 V, col 129 ones
            v_nat = [[None] * NKT for _ in range(B)]
            for b in range(B):
                for kt in range(NKT):
                    ps = psmm.tile([128, 128], BF16, tag="mm")
                    nc.tensor.transpose(
                        ps[:], vT[:, b * S + kt * 128: b * S + (kt + 1) * 128], ident_sb[:]
                    )
                    vn = vnatp.tile([128, 130], BF16, tag=f"vn{b}_{kt}")
                    nc.vector.tensor_copy(vn[:, 0:64], ps[:, 0:64])
                    nc.vector.tensor_copy(vn[:, 65:129], ps[:, 64:128])
                    nc.vector.memset(vn[:, 64:65], 1.0)
                    nc.vector.memset(vn[:, 129:130], 1.0)
                    v_nat[b][kt] = vn

            # ---------- phase 3: attention + stage for A2A; phase 4: Wo ----------
            a2a_out = []
            for b in range(B):
                a_in = dram.tile([8, 128, RPC], BF16, tag=f"a2a_in{b}")
                a_out = dram.tile([8, 128, RPC], BF16, tag=f"a2a_out{b}")
                for qc in range(NQC):
                    q_sl = slice(b * S + qc * 512, b * S + (qc + 1) * 512)
                    nkt = 4 * qc + 4
                    o_ps = [
                        psO.tile([65, 512], F32, tag=f"o{h}", name=f"o_ps{h}_{b}_{qc}")
                        for h in (0, 1)
                    ]
                    for kt in range(nkt):
                        d = 128 * (kt - 4 * qc)  # >=0 on diagonal tiles
                        lo = max(0, d)
                        k_sl = slice(b * S + kt * 128, b * S + (kt + 1) * 128)
                        for h in (0, 1):
                            hp = slice(64 * h, 64 * h + 64)
                            s_ps = psS.tile([128, 512], F32, tag=f"s{h}")
                            nc.tensor.matmul(
                                s_ps[:], kT[hp, k_sl], qT[hp, q_sl],
                                start=True, stop=True,
                            )
                            p_sb = work.tile([128, 512], BF16, tag=f"p{h}")
                            nc.scalar.activation(
                                p_sb[:, lo:512], s_ps[:, lo:512], EXP, scale=SCALE,
                            )
                            if kt >= 4 * qc:
                                hi = min(512, d + 128)
                                nc.vector.tensor_mul(
                                    p_sb[:, lo:hi], p_sb[:, lo:hi], mask_sb[:, 0:hi - lo]
                                )
                            nc.tensor.matmul(
                                o_ps[h][:, lo:512],
                                v_nat[b][kt][:, 65 * h:65 * h + 65],
                                p_sb[:, lo:512],
                                start=(kt == 0), stop=(kt == nkt - 1),
                            )
                    # normalize (per-head denominator on psum row 64) + stage
                    ot = stagep.tile([128, 512], BF16, tag="ot")
                    for h in (0, 1):
                        rc_sb = work.tile([128, 512], F32, tag="recip")
                        nc.vector.reciprocal(rc_sb[0:1, :], o_ps[h][64:65, :])
                        nc.gpsimd.partition_broadcast(
                            rc_sb[0:64, :], rc_sb[0:1, :], channels=64
                        )
                        nc.vector.tensor_mul(
                            ot[64 * h:64 * h + 64, :], o_ps[h][0:64, :], rc_sb[0:64, :]
                        )
                    nc.sync.dma_start(a_in[2 * qc], ot[:, 0:256])
                    nc.sync.dma_start(a_in[2 * qc + 1], ot[:, 256:512])
                nc.gpsimd.collective_compute(
                    "AllToAll", mybir.AluOpType.bypass, replica_groups=rg,
                    ins=[a_in.opt()], outs=[a_out.opt()],
                )
                a2a_out.append(a_out)

            for b in range(B):
                ot_sb = []
                for j in range(8):
                    t = stagep.tile([128, RPC], BF16, tag=f"og{b}_{j}")
                    nc.sync.dma_start(t[:], a2a_out[b][j])
                    ot_sb.append(t)
                for rt in range(RPC // 128):
                    r_sl = slice(rt * 128, (rt + 1) * 128)
                    o_sb = outp.tile([128, DM], F32, tag="osb")
                    for nc_i in range(2):
                        ps = psmm.tile([128, 512], F32, tag="mm")
                        for kc in range(8):
                            nc.tensor.matmul(
                                ps[:], ot_sb[kc][:, r_sl],
                                wo_sb[kc][:, nc_i * 512:(nc_i + 1) * 512],
                                start=(kc == 0), stop=(kc == 7),
                            )
                        nc.vector.tensor_copy(o_sb[:, nc_i * 512:(nc_i + 1) * 512], ps[:])
                    nc.sync.dma_start(out_ext[b, r_sl, :], o_sb[:])

    nc.compile()
    return nc


def kernel(x, Wq, bq, Wk, bk, Wv, bv, Wo):
    if "nc" not in _cache:
        _cache["nc"] = _build()
    nc = _cache["nc"]

    bf = ml_dtypes.bfloat16
    xt = np.ascontiguousarray(x.reshape(B * S, DM).T).astype(bf)
    wo_b = np.ascontiguousarray(Wo).astype(bf)
    trimask = np.triu(np.ones((128, 128), np.float32)).astype(bf)
    ident = np.eye(128, dtype=np.float32).astype(bf)

    in_maps = []
    for c in range(N_CORES):
        sl = slice(c * FPC, (c + 1) * FPC)
        in_maps.append({
            "xt": xt,
            "wq": np.ascontiguousarray(Wq[:, sl]).astype(bf),
            "wk": np.ascontiguousarray(Wk[:, sl]).astype(bf),
            "wv": np.ascontiguousarray(Wv[:, sl]).astype(bf),
            "wo": wo_b,
            "bq": np.ascontiguousarray(bq[sl]).astype(np.float32).reshape(FPC, 1),
            "bk": np.ascontiguousarray(bk[sl]).astype(np.float32).reshape(FPC, 1),
            "bv": np.ascontiguousarray(bv[sl]).astype(np.float32).reshape(FPC, 1),
            "trimask": trimask,
            "ident": ident,
        })

    trace = bool(int(os.environ.get("ATTN_KERNEL_TRACE", "0")))
    res = run_bass_kernel_spmd(nc, in_maps, core_ids=list(range(N_CORES)), trace=trace)
    if trace:
        print(f"HW exec time: {res.exec_time_ns} ns")
        _cache["exec_time_ns"] = res.exec_time_ns

    out = np.empty((B, S, DM), np.float32)
    for c in range(N_CORES):
        oc = np.asarray(res.results[c]["out"])
        for b in range(B):
            out[b, c * RPC:(c + 1) * RPC, :] = oc[b]
    return out


# revision 6
# speedup vs baseline: 1.0760x; 1.0760x over previous
"""Distributed causal multi-head attention for Trainium2 (8 NeuronCores).

Problem: B=2, S=2048, d_model=1024, 16 heads x 64 dims, causal softmax attention.

Strategy (tensor-parallel over heads + all-to-all for output projection):
  - Each core owns 2 heads (128 of the 1024 QKV features).
  - Host pre-transposes x -> X^T [1024, 4096] and casts inputs to bf16, so all
    on-chip matmuls consume feature-on-partition ("transposed") activations
    directly with no on-chip transposes of x.
  - Per core: Q^T/K^T/V^T = W^T-shard @ X^T (+bias), attention per (batch, head)
    in S^T layout ([k-partitions, q-free]) with exp (no max subtraction; scores
    are O(1) so fp32 exp is safe), causal masking via a single 128x128 upper-
    triangular mask on diagonal tiles, and denominators via an appended ones
    column on V (PE computes the partition-dim sums for free).
  - Both heads of a k-tile share one [128,1024] PSUM tile (adjacent banks) so a
    single ScalarE exp covers them; heads' S^T matmuls pack into the PE array
    via disjoint 64-row groups.
  - Normalization pre-collective (per-head denominators, fast-approx
    reciprocal), then one AllToAll per batch redistributes O^T from head-sharded
    to row-sharded; each core then computes its 2x256 output rows with full Wo.
  - Output f32; host reassembles the full [2, 2048, 1024].
"""
import os
import sys

sys.path.insert(0, "/opt/trn_rl_repo")

import numpy as np
import ml_dtypes

from concourse import bacc, mybir, tile
from concourse.bass_utils import run_bass_kernel_spmd

BF16 = mybir.dt.bfloat16
F32 = mybir.dt.float32

B, S, DM = 2, 2048, 1024
H, DK = 16, 64
N_CORES = 8
FPC = 128           # features per core = 2 heads x 64
RPC = S // N_CORES  # output rows per core per batch = 256
NKT = S // 128      # k-tiles per batch = 16
NQC = S // 512      # q-chunks per batch = 4
SCALE = 1.0 / 8.0   # 1/sqrt(64)

_cache = {}


def _build():
    nc = bacc.Bacc("TRN2", target_bir_lowering=False, debug=False, num_devices=N_CORES)

    xt = nc.dram_tensor("xt", [DM, B * S], BF16, kind="ExternalInput")
    wq = nc.dram_tensor("wq", [DM, FPC], BF16, kind="ExternalInput")
    wk = nc.dram_tensor("wk", [DM, FPC], BF16, kind="ExternalInput")
    wv = nc.dram_tensor("wv", [DM, FPC], BF16, kind="ExternalInput")
    wo = nc.dram_tensor("wo", [DM, DM], BF16, kind="ExternalInput")
    bq = nc.dram_tensor("bq", [FPC, 1], F32, kind="ExternalInput")
    bk = nc.dram_tensor("bk", [FPC, 1], F32, kind="ExternalInput")
    bv = nc.dram_tensor("bv", [FPC, 1], F32, kind="ExternalInput")
    trimask = nc.dram_tensor("trimask", [128, 128], BF16, kind="ExternalInput")
    ident = nc.dram_tensor("ident", [128, 128], BF16, kind="ExternalInput")
    out_ext = nc.dram_tensor("out", [B, RPC, DM], F32, kind="ExternalOutput")

    EXP = mybir.ActivationFunctionType.Exp
    IDENT = mybir.ActivationFunctionType.Identity
    rg = [list(range(N_CORES))]

    with tile.TileContext(nc) as tc:
        with (
            tc.tile_pool(name="xtp", bufs=1) as xtp,
            tc.tile_pool(name="wts", bufs=1) as wts,
            tc.tile_pool(name="qkv", bufs=1) as qkvp,
            tc.tile_pool(name="vnat", bufs=1) as vnatp,
            tc.tile_pool(name="work", bufs=3) as work,
            tc.tile_pool(name="stage", bufs=2) as stagep,
            tc.tile_pool(name="outp", bufs=2) as outp,
            tc.tile_pool(name="psmm", bufs=2, space="PSUM") as psmm,
            tc.tile_pool(name="psS", bufs=2, space="PSUM") as psS,
            tc.tile_pool(name="psO", bufs=1, space="PSUM") as psO,
            tc.tile_pool(name="dram", bufs=1, space="DRAM") as dram,
        ):
            # ---------- load small tensors first (unblocks first matmuls) ----------
            def load_w(w, name):
                tiles = []
                for kc in range(8):
                    t = wts.tile([128, FPC], BF16, tag=f"{name}{kc}", name=f"{name}{kc}")
                    nc.sync.dma_start(t[:], w[kc * 128:(kc + 1) * 128, :])
                    tiles.append(t)
                return tiles

            wq_sb = load_w(wq, "wq")
            wk_sb = load_w(wk, "wk")
            wv_sb = load_w(wv, "wv")
            b_sb = {}
            for name, b in (("q", bq), ("k", bk), ("v", bv)):
                t = wts.tile([FPC, 1], F32, tag=f"b{name}", name=f"b{name}")
                nc.sync.dma_start(t[:], b[:])
                b_sb[name] = t
            mask_sb = wts.tile([128, 128], BF16, tag="mask")
            nc.sync.dma_start(mask_sb[:], trimask[:])
            ident_sb = wts.tile([128, 128], BF16, tag="ident")
            nc.sync.dma_start(ident_sb[:], ident[:])
            wo_sb = []
            for kc in range(8):
                t = wts.tile([128, DM], BF16, tag=f"wo{kc}", name=f"wo{kc}")
                nc.sync.dma_start(t[:], wo[kc * 128:(kc + 1) * 128, :])
                wo_sb.append(t)

            xt_sb = []
            for kc in range(8):
                t = xtp.tile([128, B * S], BF16, tag=f"xt{kc}", name=f"xt{kc}")
                nc.sync.dma_start(t[:], xt[kc * 128:(kc + 1) * 128, :])
                xt_sb.append(t)

            # ---------- phase 1: QKV projections (transposed layout) ----------
            proj_sb = {}
            for name, w_tiles in (("q", wq_sb), ("k", wk_sb), ("v", wv_sb)):
                pt = qkvp.tile([128, B * S], BF16, tag=f"{name}T", name=f"{name}T")
                proj_sb[name] = pt
                for rc in range(8):
                    ps = psmm.tile([128, 512], F32, tag="mm", name=f"ps_{name}{rc}")
                    for kc in range(8):
                        nc.tensor.matmul(
                            ps[:], w_tiles[kc][:], xt_sb[kc][:, rc * 512:(rc + 1) * 512],
                            start=(kc == 0), stop=(kc == 7),
                        )
                    nc.scalar.activation(
                        pt[:, rc * 512:(rc + 1) * 512], ps[:], IDENT, bias=b_sb[name][:],
                    )
            qT, kT, vT = proj_sb["q"], proj_sb["k"], proj_sb["v"]

            # ---------- phase 2: V natural (+ones cols) via PE transpose ----------
            v_nat = [[None] * NKT for _ in range(B)]
            for b in range(B):
                for kt in range(NKT):
                    ps = psmm.tile([128, 128], BF16, tag="mm", name=f"pst{b}_{kt}")
                    nc.tensor.transpose(
                        ps[:], vT[:, b * S + kt * 128: b * S + (kt + 1) * 128], ident_sb[:]
                    )
                    vn = vnatp.tile([128, 130], BF16, tag=f"vn{b}_{kt}", name=f"vn{b}_{kt}")
                    nc.vector.tensor_copy(vn[:, 0:64], ps[:, 0:64])
                    nc.vector.tensor_copy(vn[:, 65:129], ps[:, 64:128])
                    nc.vector.memset(vn[:, 64:65], 1.0)
                    nc.vector.memset(vn[:, 129:130], 1.0)
                    v_nat[b][kt] = vn

            # ---------- phase 3: attention + stage for A2A ----------
            a2a_out = []
            for b in range(B):
                a_in = dram.tile([8, 128, RPC], BF16, tag=f"a2a_in{b}", name=f"a2a_in{b}")
                a_out = dram.tile([8, 128, RPC], BF16, tag=f"a2a_out{b}", name=f"a2a_out{b}")
                for qc in range(NQC):
                    q_sl = slice(b * S + qc * 512, b * S + (qc + 1) * 512)
                    nkt = 4 * qc + 4
                    o_ps = [
                        psO.tile([65, 512], F32, tag=f"o{h}", name=f"o_ps{h}_{b}_{qc}")
                        for h in (0, 1)
                    ]
                    for kt in range(nkt):
                        d = 128 * (kt - 4 * qc)  # >=0 on diagonal tiles
                        lo = max(0, d)
                        k_sl = slice(b * S + kt * 128, b * S + (kt + 1) * 128)
                        # both heads' S^T into one 2-bank psum tile
                        s_ps = psS.tile([128, 1024], F32, tag="s", name=f"s_{b}_{qc}_{kt}")
                        p_sb = work.tile([128, 1024], BF16, tag="p", name=f"p_{b}_{qc}_{kt}")
                        for h in (0, 1):
                            hp = slice(64 * h, 64 * h + 64)
                            nc.tensor.matmul(
                                s_ps[:, 512 * h:512 * h + 512],
                                kT[hp, k_sl], qT[hp, q_sl],
                                start=True, stop=True,
                            )
                        # one exp covers both heads ([lo:512+?]); head1's dead
                        # cols [512:512+lo] are computed but never read
                        nc.scalar.activation(
                            p_sb[:, lo:1024], s_ps[:, lo:1024], EXP, scale=SCALE,
                        )
                        if d >= 0:
                            hi = min(512, d + 128)
                            for h in (0, 1):
                                nc.vector.tensor_mul(
                                    p_sb[:, 512 * h + lo:512 * h + hi],
                                    p_sb[:, 512 * h + lo:512 * h + hi],
                                    mask_sb[:, 0:hi - lo],
                                )
                        for h in (0, 1):
                            nc.tensor.matmul(
                                o_ps[h][:, lo:512],
                                v_nat[b][kt][:, 65 * h:65 * h + 65],
                                p_sb[:, 512 * h + lo:512 * h + 512],
                                start=(kt == 0), stop=(kt == nkt - 1),
                            )
                    # normalize (per-head denominator on psum row 64) + stage
                    ot = stagep.tile([128, 512], BF16, tag="ot", name=f"ot{b}_{qc}")
                    for h in (0, 1):
                        rc_sb = work.tile([128, 512], F32, tag="recip", name=f"rc{b}_{qc}_{h}")
                        # custom-DVE ops need SBUF + aligned partitions: stage
                        # the denominator to sbuf row 0, then approx-recip it
                        nc.vector.tensor_copy(rc_sb[64:65, :], o_ps[h][64:65, :])
                        nc.vector.tensor_copy(rc_sb[0:1, :], rc_sb[64:65, :])
                        nc.vector.reciprocal_approx_fast(rc_sb[0:1, :], rc_sb[0:1, :])
                        nc.gpsimd.partition_broadcast(
                            rc_sb[0:64, :], rc_sb[0:1, :], channels=64
                        )
                        nc.vector.tensor_mul(
                            ot[64 * h:64 * h + 64, :], o_ps[h][0:64, :], rc_sb[0:64, :]
                        )
                    nc.sync.dma_start(a_in[2 * qc], ot[:, 0:256])
                    nc.sync.dma_start(a_in[2 * qc + 1], ot[:, 256:512])
                nc.gpsimd.collective_compute(
                    "AllToAll", mybir.AluOpType.bypass, replica_groups=rg,
                    ins=[a_in.opt()], outs=[a_out.opt()],
                )
                a2a_out.append(a_out)

            # ---------- phase 4: output projection on row shards ----------
            for b in range(B):
                ot_sb = []
                for j in range(8):
                    t = stagep.tile([128, RPC], BF16, tag=f"og{b}_{j}", name=f"og{b}_{j}")
                    nc.sync.dma_start(t[:], a2a_out[b][j])
                    ot_sb.append(t)
                for rt in range(RPC // 128):
                    r_sl = slice(rt * 128, (rt + 1) * 128)
                    o_sb = outp.tile([128, DM], F32, tag="osb", name=f"osb{b}_{rt}")
                    for nc_i in range(2):
                        ps = psmm.tile([128, 512], F32, tag="mm", name=f"pso{b}_{rt}_{nc_i}")
                        for kc in range(8):
                            nc.tensor.matmul(
                                ps[:], ot_sb[kc][:, r_sl],
                                wo_sb[kc][:, nc_i * 512:(nc_i + 1) * 512],
                                start=(kc == 0), stop=(kc == 7),
                            )
                        nc.vector.tensor_copy(o_sb[:, nc_i * 512:(nc_i + 1) * 512], ps[:])
                    nc.sync.dma_start(out_ext[b, r_sl, :], o_sb[:])

    nc.compile()
    return nc


def kernel(x, Wq, bq, Wk, bk, Wv, bv, Wo):
    if "nc" not in _cache:
        _cache["nc"] = _build()
    nc = _cache["nc"]

    bf = ml_dtypes.bfloat16
    xt = np.ascontiguousarray(np.asarray(x, np.float32).reshape(B * S, DM).T).astype(bf)
    wo_b = np.ascontiguousarray(np.asarray(Wo, np.float32)).astype(bf)
    trimask = np.triu(np.ones((128, 128), np.float32)).astype(bf)
    ident = np.eye(128, dtype=np.float32).astype(bf)

    in_maps = []
    for c in range(N_CORES):
        sl = slice(c * FPC, (c + 1) * FPC)
        in_maps.append({
            "xt": xt,
            "wq": np.ascontiguousarray(np.asarray(Wq, np.float32)[:, sl]).astype(bf),
            "wk": np.ascontiguousarray(np.asarray(Wk, np.float32)[:, sl]).astype(bf),
            "wv": np.ascontiguousarray(np.asarray(Wv, np.float32)[:, sl]).astype(bf),
            "wo": wo_b,
            "bq": np.ascontiguousarray(np.asarray(bq, np.float32)[sl]).reshape(FPC, 1),
            "bk": np.ascontiguousarray(np.asarray(bk, np.float32)[sl]).reshape(FPC, 1),
            "bv": np.ascontiguousarray(np.asarray(bv, np.float32)[sl]).reshape(FPC, 1),
            "trimask": trimask,
            "ident": ident,
        })

    trace = bool(int(os.environ.get("ATTN_KERNEL_TRACE", "0")))
    res = run_bass_kernel_spmd(nc, in_maps, core_ids=list(range(N_CORES)), trace=trace)
    if trace:
        print(f"HW exec time: {res.exec_time_ns} ns")
        _cache["exec_time_ns"] = res.exec_time_ns

    out = np.empty((B, S, DM), np.float32)
    for c in range(N_CORES):
        oc = np.asarray(res.results[c]["out"])
        for b in range(B):
            out[b, c * RPC:(c + 1) * RPC, :] = oc[b]
    return out


# revision 8
# speedup vs baseline: 1.1741x; 1.0912x over previous
"""Distributed causal multi-head attention for Trainium2 (8 NeuronCores).

Problem: B=2, S=2048, d_model=1024, 16 heads x 64 dims, causal softmax attention.

Strategy (tensor-parallel over heads + all-to-all for output projection):
  - Each core owns 2 heads (128 of the 1024 QKV features).
  - Host pre-transposes x -> X^T [1024, 4096] and casts inputs to bf16, so all
    on-chip matmuls consume feature-on-partition ("transposed") activations
    directly with no on-chip transposes of x.
  - Per core: Q^T/K^T/V^T = W^T-shard @ X^T (+bias), attention per (batch, head)
    in S^T layout ([k-partitions, q-free]) with exp (no max subtraction; scores
    are O(1) so fp32 exp is safe), causal masking via a single 128x128 upper-
    triangular mask on diagonal tiles, and denominators via an appended ones
    column on V (PE computes the partition-dim sums for free).
  - Both heads of a k-tile share one [128,1024] PSUM tile (adjacent banks) so a
    single ScalarE exp covers them; heads' S^T matmuls pack into the PE array
    via disjoint 64-row groups.
  - Normalization pre-collective (per-head denominators, fast-approx
    reciprocal), then one AllToAll per batch redistributes O^T from head-sharded
    to row-sharded; each core then computes its 2x256 output rows with full Wo.
  - Output f32; host reassembles the full [2, 2048, 1024].
"""
import os
import sys

sys.path.insert(0, "/opt/trn_rl_repo")

import numpy as np
import ml_dtypes

from concourse import bacc, mybir, tile
from concourse.tile_autobufs import add_dep_helper
from concourse.bass_utils import run_bass_kernel_spmd

BF16 = mybir.dt.bfloat16
F32 = mybir.dt.float32

B, S, DM = 2, 2048, 1024
H, DK = 16, 64
N_CORES = 8
FPC = 128           # features per core = 2 heads x 64
RPC = S // N_CORES  # output rows per core per batch = 256
NKT = S // 128      # k-tiles per batch = 16
NQC = S // 512      # q-chunks per batch = 4
SCALE = 1.0 / 8.0   # 1/sqrt(64)

_cache = {}


def _build():
    nc = bacc.Bacc("TRN2", target_bir_lowering=False, debug=False, num_devices=N_CORES)

    xt = nc.dram_tensor("xt", [DM, B * S], BF16, kind="ExternalInput")
    wq = nc.dram_tensor("wq", [DM, FPC], BF16, kind="ExternalInput")
    wk = nc.dram_tensor("wk", [DM, FPC], BF16, kind="ExternalInput")
    wv = nc.dram_tensor("wv", [DM, FPC], BF16, kind="ExternalInput")
    wo = nc.dram_tensor("wo", [DM, DM], BF16, kind="ExternalInput")
    bq = nc.dram_tensor("bq", [FPC, 1], F32, kind="ExternalInput")
    bk = nc.dram_tensor("bk", [FPC, 1], F32, kind="ExternalInput")
    bv = nc.dram_tensor("bv", [FPC, 1], F32, kind="ExternalInput")
    trimask = nc.dram_tensor("trimask", [128, 128], BF16, kind="ExternalInput")
    ident = nc.dram_tensor("ident", [128, 128], BF16, kind="ExternalInput")
    out_ext = nc.dram_tensor("out", [B, RPC, DM], F32, kind="ExternalOutput")

    EXP = mybir.ActivationFunctionType.Exp
    IDENT = mybir.ActivationFunctionType.Identity
    rg = [list(range(N_CORES))]

    with tile.TileContext(nc) as tc:
        with (
            tc.tile_pool(name="xtp", bufs=1) as xtp,
            tc.tile_pool(name="wts", bufs=1) as wts,
            tc.tile_pool(name="qkv", bufs=1) as qkvp,
            tc.tile_pool(name="vnat", bufs=1) as vnatp,
            tc.tile_pool(name="work", bufs=3) as work,
            tc.tile_pool(name="stage", bufs=2) as stagep,
            tc.tile_pool(name="outp", bufs=2) as outp,
            tc.tile_pool(name="psmm", bufs=2, space="PSUM") as psmm,
            tc.tile_pool(name="psS", bufs=2, space="PSUM") as psS,
            tc.tile_pool(name="psO", bufs=1, space="PSUM") as psO,
            tc.tile_pool(name="dram", bufs=1, space="DRAM") as dram,
        ):
            # ---------- load small tensors first (unblocks first matmuls) ----------
            def load_w(w, name):
                tiles = []
                for kc in range(8):
                    t = wts.tile([128, FPC], BF16, tag=f"{name}{kc}", name=f"{name}{kc}")
                    nc.sync.dma_start(t[:], w[kc * 128:(kc + 1) * 128, :])
                    tiles.append(t)
                return tiles

            wq_sb = load_w(wq, "wq")
            wk_sb = load_w(wk, "wk")
            wv_sb = load_w(wv, "wv")
            b_sb = {}
            for name, b in (("q", bq), ("k", bk), ("v", bv)):
                t = wts.tile([FPC, 1], F32, tag=f"b{name}", name=f"b{name}")
                nc.sync.dma_start(t[:], b[:])
                b_sb[name] = t
            mask_sb = wts.tile([128, 128], BF16, tag="mask")
            nc.sync.dma_start(mask_sb[:], trimask[:])
            ident_sb = wts.tile([128, 128], BF16, tag="ident")
            nc.sync.dma_start(ident_sb[:], ident[:])
            wo_sb = []
            for kc in range(8):
                t = wts.tile([128, DM], BF16, tag=f"wo{kc}", name=f"wo{kc}")
                nc.sync.dma_start(t[:], wo[kc * 128:(kc + 1) * 128, :])
                wo_sb.append(t)

            xt_sb = []
            for kc in range(8):
                t = xtp.tile([128, B * S], BF16, tag=f"xt{kc}", name=f"xt{kc}")
                nc.sync.dma_start(t[:], xt[kc * 128:(kc + 1) * 128, :])
                xt_sb.append(t)

            # ---------- phase 1: QKV projections (transposed layout) ----------
            proj_sb = {}
            for name, w_tiles in (("q", wq_sb), ("k", wk_sb), ("v", wv_sb)):
                pt = qkvp.tile([128, B * S], BF16, tag=f"{name}T", name=f"{name}T")
                proj_sb[name] = pt
                for rc in range(8):
                    ps = psmm.tile([128, 512], F32, tag="mm", name=f"ps_{name}{rc}")
                    for kc in range(8):
                        nc.tensor.matmul(
                            ps[:], w_tiles[kc][:], xt_sb[kc][:, rc * 512:(rc + 1) * 512],
                            start=(kc == 0), stop=(kc == 7),
                        )
                    nc.scalar.activation(
                        pt[:, rc * 512:(rc + 1) * 512], ps[:], IDENT, bias=b_sb[name][:],
                    )
            qT, kT, vT = proj_sb["q"], proj_sb["k"], proj_sb["v"]

            # ---------- phase 2: V natural (+ones cols) via PE transpose ----------
            v_nat = [[None] * NKT for _ in range(B)]
            for b in range(B):
                for kt in range(NKT):
                    ps = psmm.tile([128, 128], BF16, tag="mm", name=f"pst{b}_{kt}")
                    nc.tensor.transpose(
                        ps[:], vT[:, b * S + kt * 128: b * S + (kt + 1) * 128], ident_sb[:]
                    )
                    vn = vnatp.tile([128, 130], BF16, tag=f"vn{b}_{kt}", name=f"vn{b}_{kt}")
                    nc.vector.tensor_copy(vn[:, 0:64], ps[:, 0:64])
                    nc.vector.tensor_copy(vn[:, 65:129], ps[:, 64:128])
                    nc.vector.memset(vn[:, 64:65], 1.0)
                    nc.vector.memset(vn[:, 129:130], 1.0)
                    v_nat[b][kt] = vn

            # ---------- phase 3: attention + stage for A2A ----------
            a2a_out = []
            last_o_mm = [None, None]
            last_stage_dma = [None, None]
            for b in range(B):
                a_in = dram.tile([8, 128, RPC], BF16, tag=f"a2a_in{b}", name=f"a2a_in{b}")
                a_out = dram.tile([8, 128, RPC], BF16, tag=f"a2a_out{b}", name=f"a2a_out{b}")
                for qc in range(NQC):
                    q_sl = slice(b * S + qc * 512, b * S + (qc + 1) * 512)
                    nkt = 4 * qc + 4
                    o_ps = [
                        psO.tile([65, 512], F32, tag=f"o{h}", name=f"o_ps{h}_{b}_{qc}")
                        for h in (0, 1)
                    ]
                    for kt in range(nkt):
                        d = 128 * (kt - 4 * qc)  # >=0 on diagonal tiles
                        lo = max(0, d)
                        k_sl = slice(b * S + kt * 128, b * S + (kt + 1) * 128)
                        # both heads' S^T into one 2-bank psum tile
                        s_ps = psS.tile([128, 1024], F32, tag="s", name=f"s_{b}_{qc}_{kt}")
                        p_sb = work.tile([128, 1024], BF16, tag="p", name=f"p_{b}_{qc}_{kt}")
                        for h in (0, 1):
                            hp = slice(64 * h, 64 * h + 64)
                            nc.tensor.matmul(
                                s_ps[:, 512 * h:512 * h + 512],
                                kT[hp, k_sl], qT[hp, q_sl],
                                start=True, stop=True,
                            )
                        # one exp covers both heads ([lo:512+?]); head1's dead
                        # cols [512:512+lo] are computed but never read
                        nc.scalar.activation(
                            p_sb[:, lo:1024], s_ps[:, lo:1024], EXP, scale=SCALE,
                        )
                        if d >= 0:
                            hi = min(512, d + 128)
                            for h in (0, 1):
                                nc.vector.tensor_mul(
                                    p_sb[:, 512 * h + lo:512 * h + hi],
                                    p_sb[:, 512 * h + lo:512 * h + hi],
                                    mask_sb[:, 0:hi - lo],
                                )
                        for h in (0, 1):
                            mm = nc.tensor.matmul(
                                o_ps[h][:, lo:512],
                                v_nat[b][kt][:, 65 * h:65 * h + 65],
                                p_sb[:, 512 * h + lo:512 * h + 512],
                                start=(kt == 0), stop=(kt == nkt - 1),
                            )
                            last_o_mm[b] = mm
                    # normalize (per-head denominator on psum row 64) + stage
                    ot = stagep.tile([128, 512], BF16, tag="ot", name=f"ot{b}_{qc}")
                    for h in (0, 1):
                        rc_sb = work.tile([128, 512], F32, tag="recip", name=f"rc{b}_{qc}_{h}")
                        # custom-DVE ops need SBUF + aligned partitions: stage
                        # the denominator to sbuf row 0, then approx-recip it
                        nc.vector.tensor_copy(rc_sb[64:65, :], o_ps[h][64:65, :])
                        nc.vector.tensor_copy(rc_sb[0:1, :], rc_sb[64:65, :])
                        nc.vector.reciprocal_approx_fast(rc_sb[0:1, :], rc_sb[0:1, :])
                        nc.gpsimd.partition_broadcast(
                            rc_sb[0:64, :], rc_sb[0:1, :], channels=64
                        )
                        nc.vector.tensor_mul(
                            ot[64 * h:64 * h + 64, :], o_ps[h][0:64, :], rc_sb[0:64, :]
                        )
                    nc.sync.dma_start(a_in[2 * qc], ot[:, 0:256])
                    last_stage_dma[b] = nc.sync.dma_start(a_in[2 * qc + 1], ot[:, 256:512])
                nc.gpsimd.collective_compute(
                    "AllToAll", mybir.AluOpType.bypass, replica_groups=rg,
                    ins=[a_in.opt()], outs=[a_out.opt()],
                )
                a2a_out.append(a_out)

            # ---------- phase 4: output projection on row shards ----------
            # ordering edges (sync=False): keep phase-4 work behind batch-1
            # attention in each engine's stream, so PE never stalls on the
            # collectives mid-attention
            prev_mm = last_o_mm[1]
            prev_dma = last_stage_dma[1]
            for b in range(B):
                ot_sb = []
                for j in range(8):
                    t = stagep.tile([128, RPC], BF16, tag=f"og{b}_{j}", name=f"og{b}_{j}")
                    dma = nc.sync.dma_start(t[:], a2a_out[b][j])
                    add_dep_helper(dma.ins, prev_dma.ins, False, "phase order")
                    prev_dma = dma
                    ot_sb.append(t)
                for rt in range(RPC // 128):
                    r_sl = slice(rt * 128, (rt + 1) * 128)
                    o_sb = outp.tile([128, DM], F32, tag="osb", name=f"osb{b}_{rt}")
                    for nc_i in range(2):
                        ps = psmm.tile([128, 512], F32, tag="mm", name=f"pso{b}_{rt}_{nc_i}")
                        for kc in range(8):
                            mm = nc.tensor.matmul(
                                ps[:], ot_sb[kc][:, r_sl],
                                wo_sb[kc][:, nc_i * 512:(nc_i + 1) * 512],
                                start=(kc == 0), stop=(kc == 7),
                            )
                            add_dep_helper(mm.ins, prev_mm.ins, False, "phase order")
                            prev_mm = mm
                        nc.vector.tensor_copy(o_sb[:, nc_i * 512:(nc_i + 1) * 512], ps[:])
                    nc.sync.dma_start(out_ext[b, r_sl, :], o_sb[:])

    nc.compile()
    return nc


def kernel(x, Wq, bq, Wk, bk, Wv, bv, Wo):
    if "nc" not in _cache:
        _cache["nc"] = _build()
    nc = _cache["nc"]

    bf = ml_dtypes.bfloat16
    xt = np.ascontiguousarray(np.asarray(x, np.float32).reshape(B * S, DM).T).astype(bf)
    wo_b = np.ascontiguousarray(np.asarray(Wo, np.float32)).astype(bf)
    trimask = np.triu(np.ones((128, 128), np.float32)).astype(bf)
    ident = np.eye(128, dtype=np.float32).astype(bf)

    in_maps = []
    for c in range(N_CORES):
        sl = slice(c * FPC, (c + 1) * FPC)
        in_maps.append({
            "xt": xt,
            "wq": np.ascontiguousarray(np.asarray(Wq, np.float32)[:, sl]).astype(bf),
            "wk": np.ascontiguousarray(np.asarray(Wk, np.float32)[:, sl]).astype(bf),
            "wv": np.ascontiguousarray(np.asarray(Wv, np.float32)[:, sl]).astype(bf),
            "wo": wo_b,
            "bq": np.ascontiguousarray(np.asarray(bq, np.float32)[sl]).reshape(FPC, 1),
            "bk": np.ascontiguousarray(np.asarray(bk, np.float32)[sl]).reshape(FPC, 1),
            "bv": np.ascontiguousarray(np.asarray(bv, np.float32)[sl]).reshape(FPC, 1),
            "trimask": trimask,
            "ident": ident,
        })

    trace = bool(int(os.environ.get("ATTN_KERNEL_TRACE", "0")))
    res = run_bass_kernel_spmd(nc, in_maps, core_ids=list(range(N_CORES)), trace=trace)
    if trace:
        print(f"HW exec time: {res.exec_time_ns} ns")
        _cache["exec_time_ns"] = res.exec_time_ns

    out = np.empty((B, S, DM), np.float32)
    for c in range(N_CORES):
        oc = np.asarray(res.results[c]["out"])
        for b in range(B):
            out[b, c * RPC:(c + 1) * RPC, :] = oc[b]
    return out


# revision 10
# speedup vs baseline: 1.1898x; 1.0133x over previous
"""Distributed causal multi-head attention for Trainium2 (8 NeuronCores).

Problem: B=2, S=2048, d_model=1024, 16 heads x 64 dims, causal softmax attention.

Strategy (tensor-parallel over heads + all-to-all for output projection):
  - Each core owns 2 heads (128 of the 1024 QKV features).
  - Host pre-transposes x -> X^T [1024, 4096] and casts inputs to bf16, so all
    on-chip matmuls consume feature-on-partition ("transposed") activations
    directly with no on-chip transposes of x.
  - Per core: Q^T/K^T/V^T = W^T-shard @ X^T (+bias), attention per (batch, head)
    in S^T layout ([k-partitions, q-free]) with exp (no max subtraction; scores
    are O(1) so fp32 exp is safe), causal masking via a single 128x128 upper-
    triangular mask on diagonal tiles, and denominators via an appended ones
    column on V (PE computes the partition-dim sums for free).
  - Both heads of a k-tile share one [128,1024] PSUM tile (adjacent banks) so a
    single ScalarE exp covers them; heads' S^T matmuls pack into the PE array
    via disjoint 64-row groups.
  - Normalization pre-collective (per-head denominators, fast-approx
    reciprocal), then one AllToAll per batch redistributes O^T from head-sharded
    to row-sharded; each core then computes its 2x256 output rows with full Wo.
  - Output f32; host reassembles the full [2, 2048, 1024].
"""
import os
import sys

sys.path.insert(0, "/opt/trn_rl_repo")

import numpy as np
import ml_dtypes

from concourse import bacc, mybir, tile
from concourse.tile_autobufs import add_dep_helper
from concourse.bass_utils import run_bass_kernel_spmd

BF16 = mybir.dt.bfloat16
F32 = mybir.dt.float32

B, S, DM = 2, 2048, 1024
H, DK = 16, 64
N_CORES = 8
FPC = 128           # features per core = 2 heads x 64
RPC = S // N_CORES  # output rows per core per batch = 256
NKT = S // 128      # k-tiles per batch = 16
NQC = S // 512      # q-chunks per batch = 4
SCALE = 1.0 / 8.0   # 1/sqrt(64)

_cache = {}


def _build():
    nc = bacc.Bacc("TRN2", target_bir_lowering=False, debug=False, num_devices=N_CORES)

    xt = nc.dram_tensor("xt", [DM, B * S], BF16, kind="ExternalInput")
    wq = nc.dram_tensor("wq", [DM, FPC], BF16, kind="ExternalInput")
    wk = nc.dram_tensor("wk", [DM, FPC], BF16, kind="ExternalInput")
    wv = nc.dram_tensor("wv", [DM, FPC], BF16, kind="ExternalInput")
    wo = nc.dram_tensor("wo", [DM, DM], BF16, kind="ExternalInput")
    bq = nc.dram_tensor("bq", [FPC, 1], F32, kind="ExternalInput")
    bk = nc.dram_tensor("bk", [FPC, 1], F32, kind="ExternalInput")
    bv = nc.dram_tensor("bv", [FPC, 1], F32, kind="ExternalInput")
    trimask = nc.dram_tensor("trimask", [128, 128], BF16, kind="ExternalInput")
    ident = nc.dram_tensor("ident", [128, 128], BF16, kind="ExternalInput")
    out_ext = nc.dram_tensor("out", [B, RPC, DM], F32, kind="ExternalOutput")

    EXP = mybir.ActivationFunctionType.Exp
    IDENT = mybir.ActivationFunctionType.Identity
    rg = [list(range(N_CORES))]

    with tile.TileContext(nc) as tc:
        with (
            tc.tile_pool(name="xtp", bufs=1) as xtp,
            tc.tile_pool(name="wts", bufs=1) as wts,
            tc.tile_pool(name="qkv", bufs=1) as qkvp,
            tc.tile_pool(name="vnat", bufs=1) as vnatp,
            tc.tile_pool(name="work", bufs=3) as work,
            tc.tile_pool(name="stage", bufs=2) as stagep,
            tc.tile_pool(name="outp", bufs=2) as outp,
            tc.tile_pool(name="psmm", bufs=2, space="PSUM") as psmm,
            tc.tile_pool(name="psS", bufs=2, space="PSUM") as psS,
            tc.tile_pool(name="psO", bufs=1, space="PSUM") as psO,
            tc.tile_pool(name="dram", bufs=1, space="DRAM") as dram,
        ):
            # ---------- load small tensors first (unblocks first matmuls) ----------
            def load_w(w, name):
                tiles = []
                for kc in range(8):
                    t = wts.tile([128, FPC], BF16, tag=f"{name}{kc}", name=f"{name}{kc}")
                    nc.sync.dma_start(t[:], w[kc * 128:(kc + 1) * 128, :])
                    tiles.append(t)
                return tiles

            wq_sb = load_w(wq, "wq")
            wk_sb = load_w(wk, "wk")
            wv_sb = load_w(wv, "wv")
            b_sb = {}
            for name, b in (("q", bq), ("k", bk), ("v", bv)):
                t = wts.tile([FPC, 1], F32, tag=f"b{name}", name=f"b{name}")
                nc.sync.dma_start(t[:], b[:])
                b_sb[name] = t
            mask_sb = wts.tile([128, 128], BF16, tag="mask")
            nc.sync.dma_start(mask_sb[:], trimask[:])
            ident_sb = wts.tile([128, 128], BF16, tag="ident")
            nc.sync.dma_start(ident_sb[:], ident[:])
            wo_sb = []
            for kc in range(8):
                t = wts.tile([128, DM], BF16, tag=f"wo{kc}", name=f"wo{kc}")
                nc.sync.dma_start(t[:], wo[kc * 128:(kc + 1) * 128, :])
                wo_sb.append(t)

            xt_sb = []
            for kc in range(8):
                t = xtp.tile([128, B * S], BF16, tag=f"xt{kc}", name=f"xt{kc}")
                nc.sync.dma_start(t[:], xt[kc * 128:(kc + 1) * 128, :])
                xt_sb.append(t)

            # ---------- phase 1: QKV projections (transposed layout) ----------
            proj_sb = {}
            for name, w_tiles in (("q", wq_sb), ("k", wk_sb), ("v", wv_sb)):
                pt = qkvp.tile([128, B * S], BF16, tag=f"{name}T", name=f"{name}T")
                proj_sb[name] = pt
                for rc in range(8):
                    ps = psmm.tile([128, 512], F32, tag="mm", name=f"ps_{name}{rc}")
                    for kc in range(8):
                        nc.tensor.matmul(
                            ps[:], w_tiles[kc][:], xt_sb[kc][:, rc * 512:(rc + 1) * 512],
                            start=(kc == 0), stop=(kc == 7),
                        )
                    nc.scalar.activation(
                        pt[:, rc * 512:(rc + 1) * 512], ps[:], IDENT, bias=b_sb[name][:],
                    )
            qT, kT, vT = proj_sb["q"], proj_sb["k"], proj_sb["v"]

            # ---------- phase 2: V natural (+ones cols) via PE transpose ----------
            v_nat = [[None] * NKT for _ in range(B)]
            for b in range(B):
                for kt in range(NKT):
                    ps = psmm.tile([128, 128], BF16, tag="mm", name=f"pst{b}_{kt}")
                    nc.tensor.transpose(
                        ps[:], vT[:, b * S + kt * 128: b * S + (kt + 1) * 128], ident_sb[:]
                    )
                    vn = vnatp.tile([128, 130], BF16, tag=f"vn{b}_{kt}", name=f"vn{b}_{kt}")
                    nc.vector.tensor_copy(vn[:, 0:64], ps[:, 0:64])
                    nc.vector.tensor_copy(vn[:, 65:129], ps[:, 64:128])
                    nc.vector.memset(vn[:, 64:65], 1.0)
                    nc.vector.memset(vn[:, 129:130], 1.0)
                    v_nat[b][kt] = vn

            # ---------- phase 3: attention + stage for A2A ----------
            a2a_out = []
            last_o_mm = [None, None]
            last_stage_dma = [None, None]
            for b in range(B):
                a_in = dram.tile([8, 128, RPC], BF16, tag=f"a2a_in{b}", name=f"a2a_in{b}")
                a_out = dram.tile([8, 128, RPC], BF16, tag=f"a2a_out{b}", name=f"a2a_out{b}")
                for qc in range(NQC):
                    q_sl = slice(b * S + qc * 512, b * S + (qc + 1) * 512)
                    nkt = 4 * qc + 4
                    o_ps = [
                        psO.tile([65, 512], F32, tag=f"o{h}", name=f"o_ps{h}_{b}_{qc}")
                        for h in (0, 1)
                    ]
                    for kt in range(nkt):
                        d = 128 * (kt - 4 * qc)  # >=0 on diagonal tiles
                        lo = max(0, d)
                        k_sl = slice(b * S + kt * 128, b * S + (kt + 1) * 128)
                        # both heads' S^T into one 2-bank psum tile
                        s_ps = psS.tile([128, 1024], F32, tag="s", name=f"s_{b}_{qc}_{kt}")
                        p_sb = work.tile([128, 1024], BF16, tag="p", name=f"p_{b}_{qc}_{kt}")
                        q_lo = slice(b * S + qc * 512 + lo, b * S + (qc + 1) * 512)
                        for h in (0, 1):
                            hp = slice(64 * h, 64 * h + 64)
                            nc.tensor.matmul(
                                s_ps[:, 512 * h + lo:512 * h + 512],
                                kT[hp, k_sl], qT[hp, q_lo],
                                start=True, stop=True,
                            )
                        # one exp covers both heads ([lo:512+?]); head1's dead
                        # cols [512:512+lo] are computed but never read
                        nc.scalar.activation(
                            p_sb[:, lo:1024], s_ps[:, lo:1024], EXP, scale=SCALE,
                        )
                        if d >= 0:
                            hi = min(512, d + 128)
                            for h in (0, 1):
                                nc.vector.tensor_mul(
                                    p_sb[:, 512 * h + lo:512 * h + hi],
                                    p_sb[:, 512 * h + lo:512 * h + hi],
                                    mask_sb[:, 0:hi - lo],
                                )
                        for h in (0, 1):
                            mm = nc.tensor.matmul(
                                o_ps[h][:, lo:512],
                                v_nat[b][kt][:, 65 * h:65 * h + 65],
                                p_sb[:, 512 * h + lo:512 * h + 512],
                                start=(kt == 0), stop=(kt == nkt - 1),
                            )
                            last_o_mm[b] = mm
                    # normalize (per-head denominator on psum row 64) + stage
                    ot = stagep.tile([128, 512], BF16, tag="ot", name=f"ot{b}_{qc}")
                    for h in (0, 1):
                        rc_sb = work.tile([128, 512], F32, tag="recip", name=f"rc{b}_{qc}_{h}")
                        # custom-DVE ops need SBUF + aligned partitions: stage
                        # the denominator to sbuf row 0, then approx-recip it
                        nc.vector.tensor_copy(rc_sb[64:65, :], o_ps[h][64:65, :])
                        nc.vector.tensor_copy(rc_sb[0:1, :], rc_sb[64:65, :])
                        nc.vector.reciprocal_approx_fast(rc_sb[0:1, :], rc_sb[0:1, :])
                        nc.gpsimd.partition_broadcast(
                            rc_sb[0:64, :], rc_sb[0:1, :], channels=64
                        )
                        nc.vector.tensor_mul(
                            ot[64 * h:64 * h + 64, :], o_ps[h][0:64, :], rc_sb[0:64, :]
                        )
                    nc.sync.dma_start(a_in[2 * qc], ot[:, 0:256])
                    sdma = nc.sync.dma_start(a_in[2 * qc + 1], ot[:, 256:512])
                    if qc == NQC - 2 or last_stage_dma[b] is None:
                        last_stage_dma[b] = sdma
                nc.gpsimd.collective_compute(
                    "AllToAll", mybir.AluOpType.bypass, replica_groups=rg,
                    ins=[a_in.opt()], outs=[a_out.opt()],
                )
                a2a_out.append(a_out)

            # ---------- phase 4: output projection on row shards ----------
            # ordering edges (sync=False): keep phase-4 work behind batch-1
            # attention in each engine's stream, so PE never stalls on the
            # collectives mid-attention
            prev_mm = last_o_mm[1]
            prev_dma = last_stage_dma[1]
            for b in range(B):
                ot_sb = []
                for j in range(8):
                    t = stagep.tile([128, RPC], BF16, tag=f"og{b}_{j}", name=f"og{b}_{j}")
                    dma = nc.sync.dma_start(t[:], a2a_out[b][j])
                    add_dep_helper(dma.ins, prev_dma.ins, False, "phase order")
                    prev_dma = dma
                    ot_sb.append(t)
                for rt in range(RPC // 128):
                    r_sl = slice(rt * 128, (rt + 1) * 128)
                    o_sb = outp.tile([128, DM], F32, tag="osb", name=f"osb{b}_{rt}")
                    for nc_i in range(2):
                        ps = psmm.tile([128, 512], F32, tag="mm", name=f"pso{b}_{rt}_{nc_i}")
                        for kc in range(8):
                            mm = nc.tensor.matmul(
                                ps[:], ot_sb[kc][:, r_sl],
                                wo_sb[kc][:, nc_i * 512:(nc_i + 1) * 512],
                                start=(kc == 0), stop=(kc == 7),
                            )
                            add_dep_helper(mm.ins, prev_mm.ins, False, "phase order")
                            prev_mm = mm
                        nc.vector.tensor_copy(o_sb[:, nc_i * 512:(nc_i + 1) * 512], ps[:])
                    nc.sync.dma_start(out_ext[b, r_sl, :], o_sb[:])

    nc.compile()
    return nc


def kernel(x, Wq, bq, Wk, bk, Wv, bv, Wo):
    if "nc" not in _cache:
        _cache["nc"] = _build()
    nc = _cache["nc"]

    bf = ml_dtypes.bfloat16
    xt = np.ascontiguousarray(np.asarray(x, np.float32).reshape(B * S, DM).T).astype(bf)
    wo_b = np.ascontiguousarray(np.asarray(Wo, np.float32)).astype(bf)
    trimask = np.triu(np.ones((128, 128), np.float32)).astype(bf)
    ident = np.eye(128, dtype=np.float32).astype(bf)

    in_maps = []
    for c in range(N_CORES):
        sl = slice(c * FPC, (c + 1) * FPC)
        in_maps.append({
            "xt": xt,
            "wq": np.ascontiguousarray(np.asarray(Wq, np.float32)[:, sl]).astype(bf),
            "wk": np.ascontiguousarray(np.asarray(Wk, np.float32)[:, sl]).astype(bf),
            "wv": np.ascontiguousarray(np.asarray(Wv, np.float32)[:, sl]).astype(bf),
            "wo": wo_b,
            "bq": np.ascontiguousarray(np.asarray(bq, np.float32)[sl]).reshape(FPC, 1),
            "bk": np.ascontiguousarray(np.asarray(bk, np.float32)[sl]).reshape(FPC, 1),
            "bv": np.ascontiguousarray(np.asarray(bv, np.float32)[sl]).reshape(FPC, 1),
            "trimask": trimask,
            "ident": ident,
        })

    trace = bool(int(os.environ.get("ATTN_KERNEL_TRACE", "0")))
    res = run_bass_kernel_spmd(nc, in_maps, core_ids=list(range(N_CORES)), trace=trace)
    if trace:
        print(f"HW exec time: {res.exec_time_ns} ns")
        _cache["exec_time_ns"] = res.exec_time_ns

    out = np.empty((B, S, DM), np.float32)
    for c in range(N_CORES):
        oc = np.asarray(res.results[c]["out"])
        for b in range(B):
            out[b, c * RPC:(c + 1) * RPC, :] = oc[b]
    return out


# revision 11
# speedup vs baseline: 1.3558x; 1.1396x over previous
"""Distributed causal multi-head attention for Trainium2 (8 NeuronCores).

Problem: B=2, S=2048, d_model=1024, 16 heads x 64 dims, causal softmax attention.

Strategy (tensor-parallel over heads + all-to-all for output projection):
  - Each core owns 2 heads (128 of the 1024 QKV features).
  - Host pre-transposes x -> X^T [1024, 4096] and casts inputs to bf16, so all
    on-chip matmuls consume feature-on-partition ("transposed") activations
    directly with no on-chip transposes of x.
  - Per core: Q^T/K^T/V^T = W^T-shard @ X^T (+bias), attention per (batch, head)
    in S^T layout ([k-partitions, q-free]) with exp (no max subtraction; scores
    are O(1) so fp32 exp is safe), causal masking via a single 128x128 upper-
    triangular mask on diagonal tiles, and denominators via an appended ones
    column on V (PE computes the partition-dim sums for free).
  - Both heads of a k-tile share one [128,1024] PSUM tile (adjacent banks) so a
    single ScalarE exp covers them; heads' S^T matmuls pack into the PE array
    via disjoint 64-row groups.
  - Normalization pre-collective (per-head denominators, fast-approx
    reciprocal), then one AllToAll per batch redistributes O^T from head-sharded
    to row-sharded; each core then computes its 2x256 output rows with full Wo.
  - Output f32; host reassembles the full [2, 2048, 1024].
"""
import os
import sys

sys.path.insert(0, "/opt/trn_rl_repo")

import numpy as np
import ml_dtypes

from concourse import bacc, mybir, tile
from concourse.tile_autobufs import add_dep_helper
from concourse.bass_utils import run_bass_kernel_spmd

BF16 = mybir.dt.bfloat16
F32 = mybir.dt.float32

B, S, DM = 2, 2048, 1024
H, DK = 16, 64
N_CORES = 8
FPC = 128           # features per core = 2 heads x 64
RPC = S // N_CORES  # output rows per core per batch = 256
NKT = S // 128      # k-tiles per batch = 16
NQC = S // 512      # q-chunks per batch = 4
SCALE = 1.0 / 8.0   # 1/sqrt(64)

_cache = {}


def _build():
    nc = bacc.Bacc("TRN2", target_bir_lowering=False, debug=False, num_devices=N_CORES)

    xt = nc.dram_tensor("xt", [DM, B * S], BF16, kind="ExternalInput")
    wq = nc.dram_tensor("wq", [DM, FPC], BF16, kind="ExternalInput")
    wk = nc.dram_tensor("wk", [DM, FPC], BF16, kind="ExternalInput")
    wv = nc.dram_tensor("wv", [DM, FPC], BF16, kind="ExternalInput")
    wo = nc.dram_tensor("wo", [DM, DM], BF16, kind="ExternalInput")
    bq = nc.dram_tensor("bq", [FPC, 1], F32, kind="ExternalInput")
    bk = nc.dram_tensor("bk", [FPC, 1], F32, kind="ExternalInput")
    bv = nc.dram_tensor("bv", [FPC, 1], F32, kind="ExternalInput")
    trimask = nc.dram_tensor("trimask", [128, 128], BF16, kind="ExternalInput")
    ident = nc.dram_tensor("ident", [128, 128], BF16, kind="ExternalInput")
    out_ext = nc.dram_tensor("out", [B, RPC, DM], F32, kind="ExternalOutput")

    EXP = mybir.ActivationFunctionType.Exp
    IDENT = mybir.ActivationFunctionType.Identity
    rg = [list(range(N_CORES))]

    with tile.TileContext(nc) as tc:
        with (
            tc.tile_pool(name="xtp", bufs=1) as xtp,
            tc.tile_pool(name="wts", bufs=1) as wts,
            tc.tile_pool(name="qkv", bufs=1) as qkvp,
            tc.tile_pool(name="vnat", bufs=1) as vnatp,
            tc.tile_pool(name="work", bufs=3) as work,
            tc.tile_pool(name="stage", bufs=2) as stagep,
            tc.tile_pool(name="outp", bufs=2) as outp,
            tc.tile_pool(name="psmm", bufs=2, space="PSUM") as psmm,
            tc.tile_pool(name="psS", bufs=2, space="PSUM") as psS,
            tc.tile_pool(name="psO", bufs=1, space="PSUM") as psO,
            tc.tile_pool(name="dram", bufs=1, space="DRAM") as dram,
        ):
            # ---------- load small tensors first (unblocks first matmuls) ----------
            def load_w(w, name):
                tiles = []
                for kc in range(8):
                    t = wts.tile([128, FPC], BF16, tag=f"{name}{kc}", name=f"{name}{kc}")
                    nc.sync.dma_start(t[:], w[kc * 128:(kc + 1) * 128, :])
                    tiles.append(t)
                return tiles

            wq_sb = load_w(wq, "wq")
            wk_sb = load_w(wk, "wk")
            wv_sb = load_w(wv, "wv")
            b_sb = {}
            for name, b in (("q", bq), ("k", bk), ("v", bv)):
                t = wts.tile([FPC, 1], F32, tag=f"b{name}", name=f"b{name}")
                nc.sync.dma_start(t[:], b[:])
                b_sb[name] = t
            mask_sb = wts.tile([128, 128], BF16, tag="mask")
            nc.sync.dma_start(mask_sb[:], trimask[:])
            ident_sb = wts.tile([128, 128], BF16, tag="ident")
            nc.sync.dma_start(ident_sb[:], ident[:])
            wo_sb = []
            for kc in range(8):
                t = wts.tile([128, DM], BF16, tag=f"wo{kc}", name=f"wo{kc}")
                nc.sync.dma_start(t[:], wo[kc * 128:(kc + 1) * 128, :])
                wo_sb.append(t)

            xt_sb = []
            for kc in range(8):
                t = xtp.tile([128, B * S], BF16, tag=f"xt{kc}", name=f"xt{kc}")
                nc.sync.dma_start(t[:], xt[kc * 128:(kc + 1) * 128, :])
                xt_sb.append(t)

            # ---------- phases 1-3 interleaved ----------
            # row-chunk rc feeds q-chunk (b, qi): projections for rc, then V
            # transposes for that q range, then attention for (b, qi). This
            # overlaps ScalarE exp work with TensorE projection matmuls.
            proj_sb = {}
            for name in ("q", "k", "v"):
                proj_sb[name] = qkvp.tile(
                    [128, B * S], BF16, tag=f"{name}T", name=f"{name}T"
                )
            qT, kT, vT = proj_sb["q"], proj_sb["k"], proj_sb["v"]
            w_by_name = {"q": wq_sb, "k": wk_sb, "v": wv_sb}
            v_nat = [[None] * NKT for _ in range(B)]
            a2a_out = []
            a_in = [None, None]
            last_o_mm = [None, None]
            last_stage_dma = [None, None]

            for b in range(B):
                a_in[b] = dram.tile([8, 128, RPC], BF16, tag=f"a2a_in{b}", name=f"a2a_in{b}")

            for rc in range(8):
                b, qc = (0, rc) if rc < 4 else (1, rc - 4)
                # projections for this row chunk
                for name in ("q", "k", "v"):
                    ps = psmm.tile([128, 512], F32, tag="mm", name=f"ps_{name}{rc}")
                    for kc in range(8):
                        nc.tensor.matmul(
                            ps[:], w_by_name[name][kc][:],
                            xt_sb[kc][:, rc * 512:(rc + 1) * 512],
                            start=(kc == 0), stop=(kc == 7),
                        )
                    nc.scalar.activation(
                        proj_sb[name][:, rc * 512:(rc + 1) * 512], ps[:], IDENT,
                        bias=b_sb[name][:],
                    )
                # V natural (+ones cols) for this q range
                for kt in range(4 * qc, 4 * qc + 4):
                    ps = psmm.tile([128, 128], BF16, tag="mm", name=f"pst{b}_{kt}")
                    nc.tensor.transpose(
                        ps[:], vT[:, b * S + kt * 128: b * S + (kt + 1) * 128],
                        ident_sb[:],
                    )
                    vn = vnatp.tile([128, 130], BF16, tag=f"vn{b}_{kt}", name=f"vn{b}_{kt}")
                    nc.vector.tensor_copy(vn[:, 0:64], ps[:, 0:64])
                    nc.vector.tensor_copy(vn[:, 65:129], ps[:, 64:128])
                    nc.vector.memset(vn[:, 64:65], 1.0)
                    nc.vector.memset(vn[:, 129:130], 1.0)
                    v_nat[b][kt] = vn
                # attention for (b, qc)
                q_sl = slice(b * S + qc * 512, b * S + (qc + 1) * 512)
                nkt = 4 * qc + 4
                o_ps = [
                    psO.tile([65, 512], F32, tag=f"o{h}", name=f"o_ps{h}_{b}_{qc}")
                    for h in (0, 1)
                ]
                for kt in range(nkt):
                    d = 128 * (kt - 4 * qc)  # >=0 on diagonal tiles
                    lo = max(0, d)
                    k_sl = slice(b * S + kt * 128, b * S + (kt + 1) * 128)
                    s_ps = psS.tile([128, 1024], F32, tag="s", name=f"s_{b}_{qc}_{kt}")
                    p_sb = work.tile([128, 1024], BF16, tag="p", name=f"p_{b}_{qc}_{kt}")
                    q_lo = slice(b * S + qc * 512 + lo, b * S + (qc + 1) * 512)
                    for h in (0, 1):
                        hp = slice(64 * h, 64 * h + 64)
                        nc.tensor.matmul(
                            s_ps[:, 512 * h + lo:512 * h + 512],
                            kT[hp, k_sl], qT[hp, q_lo],
                            start=True, stop=True,
                        )
                    nc.scalar.activation(
                        p_sb[:, lo:1024], s_ps[:, lo:1024], EXP, scale=SCALE,
                    )
                    if d >= 0:
                        hi = min(512, d + 128)
                        for h in (0, 1):
                            nc.vector.tensor_mul(
                                p_sb[:, 512 * h + lo:512 * h + hi],
                                p_sb[:, 512 * h + lo:512 * h + hi],
                                mask_sb[:, 0:hi - lo],
                            )
                    for h in (0, 1):
                        mm = nc.tensor.matmul(
                            o_ps[h][:, lo:512],
                            v_nat[b][kt][:, 65 * h:65 * h + 65],
                            p_sb[:, 512 * h + lo:512 * h + 512],
                            start=(kt == 0), stop=(kt == nkt - 1),
                        )
                        last_o_mm[b] = mm
                # normalize (per-head denominator on psum row 64) + stage
                ot = stagep.tile([128, 512], BF16, tag="ot", name=f"ot{b}_{qc}")
                for h in (0, 1):
                    rc_sb = work.tile([128, 512], F32, tag="recip", name=f"rc{b}_{qc}_{h}")
                    nc.vector.tensor_copy(rc_sb[64:65, :], o_ps[h][64:65, :])
                    nc.vector.tensor_copy(rc_sb[0:1, :], rc_sb[64:65, :])
                    nc.vector.reciprocal_approx_fast(rc_sb[0:1, :], rc_sb[0:1, :])
                    nc.gpsimd.partition_broadcast(
                        rc_sb[0:64, :], rc_sb[0:1, :], channels=64
                    )
                    nc.vector.tensor_mul(
                        ot[64 * h:64 * h + 64, :], o_ps[h][0:64, :], rc_sb[0:64, :]
                    )
                nc.sync.dma_start(a_in[b][2 * qc], ot[:, 0:256])
                sdma = nc.sync.dma_start(a_in[b][2 * qc + 1], ot[:, 256:512])
                if qc == NQC - 2 or last_stage_dma[b] is None:
                    last_stage_dma[b] = sdma
                if qc == NQC - 1:
                    a_out = dram.tile(
                        [8, 128, RPC], BF16, tag=f"a2a_out{b}", name=f"a2a_out{b}"
                    )
                    nc.gpsimd.collective_compute(
                        "AllToAll", mybir.AluOpType.bypass, replica_groups=rg,
                        ins=[a_in[b].opt()], outs=[a_out.opt()],
                    )
                    a2a_out.append(a_out)

            # ---------- phase 4: output projection on row shards ----------
            # ordering edges (sync=False): keep phase-4 work behind batch-1
            # attention in each engine's stream, so PE never stalls on the
            # collectives mid-attention
            prev_mm = last_o_mm[1]
            prev_dma = last_stage_dma[1]
            for b in range(B):
                ot_sb = []
                for j in range(8):
                    t = stagep.tile([128, RPC], BF16, tag=f"og{b}_{j}", name=f"og{b}_{j}")
                    dma = nc.sync.dma_start(t[:], a2a_out[b][j])
                    add_dep_helper(dma.ins, prev_dma.ins, False, "phase order")
                    prev_dma = dma
                    ot_sb.append(t)
                for rt in range(RPC // 128):
                    r_sl = slice(rt * 128, (rt + 1) * 128)
                    o_sb = outp.tile([128, DM], F32, tag="osb", name=f"osb{b}_{rt}")
                    for nc_i in range(2):
                        ps = psmm.tile([128, 512], F32, tag="mm", name=f"pso{b}_{rt}_{nc_i}")
                        for kc in range(8):
                            mm = nc.tensor.matmul(
                                ps[:], ot_sb[kc][:, r_sl],
                                wo_sb[kc][:, nc_i * 512:(nc_i + 1) * 512],
                                start=(kc == 0), stop=(kc == 7),
                            )
                            add_dep_helper(mm.ins, prev_mm.ins, False, "phase order")
                            prev_mm = mm
                        nc.vector.tensor_copy(o_sb[:, nc_i * 512:(nc_i + 1) * 512], ps[:])
                    nc.sync.dma_start(out_ext[b, r_sl, :], o_sb[:])

    nc.compile()
    return nc


def kernel(x, Wq, bq, Wk, bk, Wv, bv, Wo):
    if "nc" not in _cache:
        _cache["nc"] = _build()
    nc = _cache["nc"]

    bf = ml_dtypes.bfloat16
    xt = np.ascontiguousarray(np.asarray(x, np.float32).reshape(B * S, DM).T).astype(bf)
    wo_b = np.ascontiguousarray(np.asarray(Wo, np.float32)).astype(bf)
    trimask = np.triu(np.ones((128, 128), np.float32)).astype(bf)
    ident = np.eye(128, dtype=np.float32).astype(bf)

    in_maps = []
    for c in range(N_CORES):
        sl = slice(c * FPC, (c + 1) * FPC)
        in_maps.append({
            "xt": xt,
            "wq": np.ascontiguousarray(np.asarray(Wq, np.float32)[:, sl]).astype(bf),
            "wk": np.ascontiguousarray(np.asarray(Wk, np.float32)[:, sl]).astype(bf),
            "wv": np.ascontiguousarray(np.asarray(Wv, np.float32)[:, sl]).astype(bf),
            "wo": wo_b,
            "bq": np.ascontiguousarray(np.asarray(bq, np.float32)[sl]).reshape(FPC, 1),
            "bk": np.ascontiguousarray(np.asarray(bk, np.float32)[sl]).reshape(FPC, 1),
            "bv": np.ascontiguousarray(np.asarray(bv, np.float32)[sl]).reshape(FPC, 1),
            "trimask": trimask,
            "ident": ident,
        })

    trace = bool(int(os.environ.get("ATTN_KERNEL_TRACE", "0")))
    res = run_bass_kernel_spmd(nc, in_maps, core_ids=list(range(N_CORES)), trace=trace)
    if trace:
        print(f"HW exec time: {res.exec_time_ns} ns")
        _cache["exec_time_ns"] = res.exec_time_ns

    out = np.empty((B, S, DM), np.float32)
    for c in range(N_CORES):
        oc = np.asarray(res.results[c]["out"])
        for b in range(B):
            out[b, c * RPC:(c + 1) * RPC, :] = oc[b]
    return out
